# revision 42
# baseline (speedup 1.0000x reference)
"""Distributed Trainium2 kernel for a single causal attention head.

Problem (hardcoded): B=4, S=2048, D_MODEL=1024, HEAD_DIM=64, fp32 inputs.
    q = query @ Wq + bq ; k = key @ Wk + bk ; v = value @ Wv + bv
    scores = q k^T / sqrt(H) ; masked softmax ; out = att @ v

Sharding (8 NeuronCores): core c = (b, h) with b = c//2, h = c%2.
Each core owns 4 query chunks of 256 rows of batch b and projects the
full k/v of its batch locally.  To balance causal work with one SPMD
program, chunks are assigned h=0 -> {0,3,4,7}, h=1 -> {1,2,5,6}.  The
program works in two slot-PAIRS of 512 query rows; pair p computes
j-tiles [0, SHARED_p) at full 512 width and j-tiles [SHARED_p, SOLO_p)
at 256 width (second chunk only).  Causal: SHARED=(4,12), SOLO=(8,16).
Per-core differences are pure data: gathered query rows and host-built
predicate masks (from the real `mask` input) that zero attention
weights after exp; predicates exist for j-tiles [8p, 8p+8) of pair p.

Device layout trick: query/key/value shards are passed TRANSPOSED and
pre-packed ([128, D/128, S] bf16, one contiguous DMA line per
partition) so every matmul contracts over the partition dim with
operands in natural layout (no on-device input transposes):
  qT[h,i]    = Wq^T Xq^T     (lhsT=Wq chunk, rhs=XqT chunk)
  kT/vT[h,j] = W^T X^T
  v[j,h]     = vT via DRAM bounce + DMA-transpose (2-byte dtype)
  sT[j,i]    = kT-tile as lhsT, rhs=qT         (scores transposed)
  att        = exp(sT * 0.125)   (ScalarE, PSUM->SBUF, bf16)
  oT[65,i]  += v_aug-tile as lhsT, rhs=att     (v_aug ones column ->
                                                row 64 = softmax denom)
Final: PE-transpose [65,128] blocks, scale by reciprocal of column 64,
DMA out as [i, 64] fp32.

DMA discipline (the earlier versions' bottleneck): big wait-free input
DMAs on the sync HWDGE ring in dependency order; the v bounce +
transposes ride the scalar HWDGE ring; output leaves via gpsimd SWDGE.
"""

import os

import numpy as np
import ml_dtypes

import concourse.bass as bass
import concourse.tile as tile
from concourse import bacc, mybir
from concourse.bass import ds
from concourse.bass_utils import run_bass_kernel_spmd
from concourse.masks import make_identity

B, S, D, H = 4, 2048, 1024, 64
P = 128
NCORES = 8
CHUNK = 256               # query rows per chunk/slot
NSLOTS = 4
NPAIRS = 2                # slot pairs; 512 q rows each
NQ = NSLOTS * CHUNK       # 1024
JT = S // P               # 16 j-tiles of 128 keys
DCH = D // P              # 8 contraction chunks
FP = mybir.dt.float32
BF = mybir.dt.bfloat16
U8 = mybir.dt.uint8
BF_NP = ml_dtypes.bfloat16

# (shared extent, solo extent) per pair, in j-tiles; slot extents derive
CAUSAL_PAIRS = ((4, 8), (12, 16))
CAUSAL_CHUNKS = {0: (0, 3, 4, 7), 1: (1, 2, 5, 6)}
# predicate entries: (pair, jt) for jt in [8p, 8p+8)
CAUSAL_MASKED = [(p, jt) for p in range(2) for jt in range(8 * p, 8 * p + 8)]

FULL_PAIRS = ((16, 16), (16, 16))
FULL_CHUNKS = {0: (0, 1, 2, 3), 1: (4, 5, 6, 7)}
FULL_MASKED = [(p, jt) for p in range(2) for jt in range(16)]

LAST_RESULTS = None
_PROGRAM_CACHE = {}


def _build_program(pairs, masked_slots, devpred):
    """Build the SPMD Bass program (identical on all 8 cores).

    devpred=True generates the causal predicate masks on-device from a
    tiny per-core threshold table instead of a 1 MB host predicate DMA.
    """
    nc = bacc.Bacc("TRN2", target_bir_lowering=False, debug=False,
                   num_devices=NCORES)

    qT_d = nc.dram_tensor("qT", [P, DCH, NQ], BF, kind="ExternalInput").ap()
    kT_d = nc.dram_tensor("kT", [P, DCH, S], BF, kind="ExternalInput").ap()
    vT_d = nc.dram_tensor("vT", [P, DCH, S], BF, kind="ExternalInput").ap()
    wall_d = nc.dram_tensor("wall", [P, DCH, 3 * H], BF,
                            kind="ExternalInput").ap()
    ball_d = nc.dram_tensor("ball", [H, 3], FP, kind="ExternalInput").ap()
    nmask = len(masked_slots)
    if devpred:
        thr_d = nc.dram_tensor("thr", [P, nmask, 2], FP,
                               kind="ExternalInput").ap()
    else:
        pred_d = nc.dram_tensor("pred", [P, nmask, 2 * CHUNK], BF,
                                kind="ExternalInput").ap()
    out_d = nc.dram_tensor("out", [NQ, H], FP, kind="ExternalOutput").ap()
    debug = bool(os.environ.get("BASS_DEBUG_DUMP"))
    if debug:
        dbg_k = nc.dram_tensor("dbg_k", [P, S], BF, kind="ExternalOutput").ap()
        dbg_v = nc.dram_tensor("dbg_v", [P, JT, H + 1], BF,
                               kind="ExternalOutput").ap()
        dbg_q = nc.dram_tensor("dbg_q", [P, NQ], BF,
                               kind="ExternalOutput").ap()

    with tile.TileContext(nc) as tc:
        with (
            tc.tile_pool(name="const", bufs=1) as const,
            tc.tile_pool(name="resident", bufs=1) as res,
            tc.tile_pool(name="attp", bufs=26) as attp,
            tc.tile_pool(name="outp", bufs=2) as outp,
            tc.tile_pool(name="dram", bufs=1, space="DRAM") as dram,
            tc.tile_pool(name="pp", bufs=2, space="PSUM") as pp,
            tc.tile_pool(name="psc", bufs=2, space="PSUM") as psc,
            tc.tile_pool(name="pout", bufs=2, space="PSUM") as pout,
            tc.tile_pool(name="ptr", bufs=2, space="PSUM") as ptr,
        ):
            # ---- constants: 2 DMAs on the scalar ring ----
            wall_sb = const.tile([P, DCH, 3 * H], BF, tag="wall")
            nc.scalar.dma_start(wall_sb, wall_d)
            ball_sb = const.tile([H, 3], FP, tag="ball")
            nc.scalar.dma_start(ball_sb, ball_d)
            wk_sb = wall_sb[:, :, 0:H]
            wv_sb = wall_sb[:, :, H:2 * H]
            wq_sb = wall_sb[:, :, 2 * H:3 * H]
            bk_sb = ball_sb[:, 0:1]
            bv_sb = ball_sb[:, 1:2]
            bq_sb = ball_sb[:, 2:3]
            zeros_sb = const.tile([P, 2 * CHUNK], BF, tag="zeros")
            nc.vector.memset(zeros_sb, 0.0)
            ident = const.tile([P, P], FP, tag="ident")
            make_identity(nc, ident)
            identb = const.tile([P, P], BF, tag="identb")
            make_identity(nc, identb)

            # ---- input DMAs on the sync ring, dependency order:
            # k cols 0:1024, q pair0, pred, v cols 0:1024,
            # k cols 1024:2048, v cols 1024:2048, q pair1
            xk_sb = res.tile([P, DCH, S], BF, tag="xk")
            xv_sb = res.tile([P, DCH, S], BF, tag="xv")
            xq_sb = res.tile([P, DCH, NQ], BF, tag="xq")
            pred_sb = res.tile([P, nmask, 2 * CHUNK], BF, tag="pred")

            def dma_cols(eng, dst, src, c0, c1):
                eng.dma_start(dst[:, :, ds(c0, c1 - c0)],
                              src[:, :, ds(c0, c1 - c0)])

            # ONE ring (sync), strict dependency order: transfers drain
            # serially at full HBM rate, so the critical k/q path lands
            # first instead of round-robin-sharing with v.
            dma_cols(nc.sync, xk_sb, kT_d, 0, S // 2)
            dma_cols(nc.sync, xq_sb, qT_d, 0, NQ // 2)
            dma_cols(nc.sync, xq_sb, qT_d, NQ // 2, NQ)
            dma_cols(nc.sync, xv_sb, vT_d, 0, S // 2)
            dma_cols(nc.sync, xk_sb, kT_d, S // 2, S)
            dma_cols(nc.sync, xv_sb, vT_d, S // 2, S)

            if devpred:
                # generate predicates on-device: pred[p, mi, h*256+f] =
                # (f < thr[p, mi, h])
                thr_sb = const.tile([P, nmask, 2], FP, tag="thr")
                nc.scalar.dma_start(thr_sb, thr_d)
                iota_sb = const.tile([P, CHUNK], FP, tag="iota")
                nc.gpsimd.iota(iota_sb, pattern=[[1, CHUNK]], base=0,
                               channel_multiplier=0,
                               allow_small_or_imprecise_dtypes=True)
                for mi in range(nmask):
                    for half in range(2):
                        nc.gpsimd.tensor_scalar(
                            pred_sb[:, mi, ds(half * CHUNK, CHUNK)],
                            iota_sb, thr_sb[:, mi, ds(half, 1)], None,
                            mybir.AluOpType.is_ge)
            else:
                nc.sync.dma_start(pred_sb, pred_d)

            # ---- PE warm-up: keep HAM at full clock until k data lands
            WARM_MMS = 16
            pwarm = pp.tile([P, 512], FP, tag="pp", name="pwarm")
            for _ in range(WARM_MMS):
                nc.tensor.matmul(pwarm, lhsT=identb,
                                 rhs=zeros_sb, start=True, stop=True)

            # col-tiled projection pair: two M=64 matmuls run concurrently
            # in PE column groups 0 / 1, contracting the same weight over
            # two different 512-wide input chunks.
            def proj_pair(w_sb, x_sb, c0, bias, out_fn, name):
                pj = pp.tile([P, 512], FP, tag="pp", name=name)
                for d in range(DCH):
                    nc.tensor.matmul(pj[0:H, :], lhsT=w_sb[:, d, :],
                                     rhs=x_sb[:, d, ds(c0, 512)],
                                     start=(d == 0), stop=(d == DCH - 1),
                                     skip_group_check=True)
                    nc.tensor.matmul(pj[H:2 * H, :], lhsT=w_sb[:, d, :],
                                     rhs=x_sb[:, d, ds(c0 + 512, 512)],
                                     start=(d == 0), stop=(d == DCH - 1),
                                     tile_position=(0, H),
                                     skip_group_check=True)
                out_fn(pj[0:H, :], c0)
                out_fn(pj[H:2 * H, :], c0 + 512)

            k_sb = res.tile([P, S], BF, tag="k")
            nc.vector.memset(k_sb[H:, :], 0.0)
            vT_sb = res.tile([P, S], BF, tag="vT")
            v_sb = res.tile([P, JT, H + 1], BF, tag="v")
            q_sb = res.tile([P, NQ], BF, tag="q")
            nc.vector.memset(q_sb[H:, :], 0.0)

            # projection epilogues ride DVE (tensor_scalar_add with a
            # per-partition bias), keeping ACT free for the exp chain
            def k_out(pj, c0):
                nc.vector.tensor_scalar_add(k_sb[:H, ds(c0, 512)], pj,
                                            bk_sb)

            def v_out(pj, c0):
                nc.vector.tensor_scalar_add(vT_sb[:H, ds(c0, 512)], pj,
                                            bv_sb)
                for jt in range(c0 // P, c0 // P + 4):
                    pvt = ptr.tile([P, P], BF, tag="tr", name="pvt")
                    nc.tensor.transpose(pvt, vT_sb[:, ds(jt * P, P)], identb)
                    nc.vector.tensor_copy(v_sb[:, jt, 0:H], pvt[:, :H])
                    nc.vector.memset(v_sb[:, jt, H:], 1.0)

            def q_out(pj, c0):
                nc.vector.tensor_scalar_add(q_sb[:H, ds(c0, 512)], pj,
                                            bq_sb)

            mask_idx = {sj: i for i, sj in enumerate(masked_slots)}
            W = 2 * CHUNK  # 512
            po_tiles = {}
            att_tiles = {}

            def emit_score(pr, jt):
                shared, solo = pairs[pr]
                wide = jt < shared
                c0 = pr * W if wide else pr * W + CHUNK
                n = W if wide else CHUNK
                ps = psc.tile([P, n], FP, tag="sc", name="ps")
                nc.tensor.matmul(ps, lhsT=k_sb[:, ds(jt * P, P)],
                                 rhs=q_sb[:, ds(c0, n)],
                                 start=True, stop=True)
                att = attp.tile([P, n], BF, tag="att", name="att")
                nc.scalar.activation(att, ps,
                                     mybir.ActivationFunctionType.Exp,
                                     scale=0.125)
                mi = mask_idx.get((pr, jt))
                if mi is not None:
                    off = 0 if wide else CHUNK
                    nc.vector.tensor_mul(
                        att, att, pred_sb[:, mi, ds(off, n)])
                att_tiles[(pr, jt)] = (att, c0, n)

            def emit_av(pr, jt):
                solo = pairs[pr][1]
                if pr not in po_tiles:
                    po_tiles[pr] = pout.tile([H + 1, W], FP, tag="po",
                                             name=f"po{pr}")
                att, c0, n = att_tiles.pop((pr, jt))
                nc.tensor.matmul(po_tiles[pr][:, ds(c0 - pr * W, n)],
                                 lhsT=v_sb[:, jt, :], rhs=att,
                                 start=(jt == 0), stop=(jt == solo - 1),
                                 skip_group_check=True)

            out_stage = res.tile([P, NQ // P, H], FP, tag="ostage")

            def epilogue(pr):
                po = po_tiles[pr]
                # transpose + normalize into the staging tile
                oT_sb = outp.tile([P, W], FP, tag="oT")
                nc.vector.tensor_copy(oT_sb[:H + 1, :], po)
                for t in range(W // P):
                    pt = ptr.tile([P, P], FP, tag="tr")
                    nc.tensor.transpose(pt, oT_sb[:, ds(t * P, P)], ident)
                    recip = outp.tile([P, 1], FP, tag="recip")
                    nc.vector.reciprocal(recip, pt[:, H:H + 1])
                    nc.vector.tensor_scalar_mul(
                        out_stage[:, pr * (W // P) + t, :], pt[:, :H], recip)

            # Decoupled emission schedule (per-engine FIFOs are in-order;
            # insert each group where its data will have arrived):
            # fill order: k01@~14, q@~20, k23@~26, v01@~32, v23@~38
            def q_solo(pr):
                pq = pp.tile([H, 512], FP, tag="pp", name="pq")
                for d in range(DCH):
                    nc.tensor.matmul(pq, lhsT=wq_sb[:, d, :],
                                     rhs=xq_sb[:, d, ds(pr * 512, 512)],
                                     start=(d == 0), stop=(d == DCH - 1))
                q_out(pq, pr * 512)

            proj_pair(wk_sb, xk_sb, 0, bk_sb, k_out, "pk01")
            for _ in range(7):    # bridge PE idle until q arrives
                nc.tensor.matmul(pwarm, lhsT=identb, rhs=zeros_sb,
                                 start=True, stop=True)
            q_solo(0)
            q_solo(1)
            # scores for pair0 + pair1's low half, ACT-paced; second-half
            # k projection slots in once k23 has landed
            # fully decoupled: scores+exp groups placed at data-arrival
            # FIFO positions; avs (pure PE, gated on v_sb) fill the gaps
            solo0, solo1 = pairs[0][1], pairs[1][1]
            for jt in range(solo0):
                emit_score(0, jt)
            proj_pair(wv_sb, xv_sb, 0, bv_sb, v_out, "pv01")
            for jt in range(8):
                emit_score(1, jt)
            for jt in range(solo0):
                emit_av(0, jt)
            proj_pair(wk_sb, xk_sb, S // 2, bk_sb, k_out, "pk23")
            for jt in range(8, solo1):
                emit_score(1, jt)
            epilogue(0)
            for jt in range(8):
                emit_av(1, jt)
            proj_pair(wv_sb, xv_sb, S // 2, bv_sb, v_out, "pv23")
            for jt in range(8, solo1):
                emit_av(1, jt)
            epilogue(1)
            nc.gpsimd.dma_start(
                out_d.rearrange("(t p) h -> p t h", p=P), out_stage)

            if debug:
                nc.gpsimd.dma_start(dbg_k, k_sb)
                nc.gpsimd.dma_start(dbg_v, v_sb)
                nc.gpsimd.dma_start(dbg_q, q_sb)

    nc.compile()
    return nc


def _slot_extents(pairs):
    return (pairs[0][0], pairs[0][1], pairs[1][0], pairs[1][1])


def _mask_fits_causal_variant(mask):
    """Causal variant is valid iff, for every chunk, nothing is allowed
    beyond its computed bound and everything below its predicate region
    is allowed."""
    extents = _slot_extents(CAUSAL_PAIRS)
    for h, chunks in CAUSAL_CHUNKS.items():
        for s, g in enumerate(chunks):
            rows = slice(g * CHUNK, (g + 1) * CHUNK)
            bound = extents[s] * P
            lo = (8 * (s // 2)) * P  # predicates cover [8p, 8p+8)
            if bound < S and mask[:, rows, bound:].any():
                return False
            if lo > 0 and not mask[:, rows, :lo].all():
                return False
    return True


def _pack(xT):
    """[D, S] -> [128, D/128, S]: one contiguous DMA line/partition."""
    d, s = xT.shape
    return np.ascontiguousarray(
        xT.reshape(DCH, P, s).transpose(1, 0, 2)).astype(BF_NP)


def _np_reference(query, key, value, mask, Wq, bq, Wk, bk, Wv, bv):
    q = query @ Wq + bq
    k = key @ Wk + bk
    v = value @ Wv + bv
    scores = np.einsum("bqh,bkh->bqk", q, k) / np.sqrt(np.float32(H))
    scores = np.where(mask, scores, np.float32(-1e9))
    scores -= scores.max(axis=-1, keepdims=True)
    e = np.exp(scores)
    att = e / e.sum(axis=-1, keepdims=True)
    return np.einsum("bqk,bkh->bqh", att, v).astype(np.float32)


def kernel(query, key, value, mask, Wq, bq, Wk, bk, Wv, bv):
    global LAST_RESULTS
    query = np.asarray(query, dtype=np.float32)
    key = np.asarray(key, dtype=np.float32)
    value = np.asarray(value, dtype=np.float32)
    mask = np.asarray(mask).astype(bool)
    Wq = np.asarray(Wq, dtype=np.float32)
    Wk = np.asarray(Wk, dtype=np.float32)
    Wv = np.asarray(Wv, dtype=np.float32)
    bq = np.asarray(bq, dtype=np.float32)
    bk = np.asarray(bk, dtype=np.float32)
    bv = np.asarray(bv, dtype=np.float32)

    tril = np.tril(np.ones((S, S), dtype=bool))
    devpred = all(np.array_equal(mask[b], tril) for b in range(B))
    if not devpred:
        # non-causal masks never occur for this problem; fall back to an
        # exact host implementation rather than an untested device path
        return _np_reference(query, key, value, mask, Wq, bq, Wk, bk,
                             Wv, bv)
    pairs, chunks_of, masked = CAUSAL_PAIRS, CAUSAL_CHUNKS, CAUSAL_MASKED
    key_v = ("causal", True)

    if key_v not in _PROGRAM_CACHE:
        _PROGRAM_CACHE[key_v] = _build_program(pairs, masked, True)
    nc = _PROGRAM_CACHE[key_v]

    def packw(w):
        return np.ascontiguousarray(
            w.reshape(DCH, P, H).transpose(1, 0, 2)).astype(BF_NP)

    # weight layout must match the wall_sb slicing: wk | wv | wq
    wall_in = np.concatenate([packw(Wk), packw(Wv), packw(Wq)], axis=2)
    wall_in = np.ascontiguousarray(wall_in)
    ball_in = np.ascontiguousarray(
        np.stack([bk, bv, bq], axis=1).astype(np.float32))

    in_maps = []
    for c in range(NCORES):
        b, h = divmod(c, 2)
        chunks = chunks_of[h]
        q_rows = np.concatenate(
            [query[b, g * CHUNK:(g + 1) * CHUNK, :] for g in chunks], axis=0)
        qT = _pack(q_rows.T)
        kT = _pack(key[b].T)
        vT = _pack(value[b].T)
        im = {"qT": qT, "kT": kT, "vT": vT,
              "wall": wall_in, "ball": ball_in}
        if devpred:
            # threshold table: pred[p, mi, half*256+f] = (f < thr)
            thr = np.zeros((P, len(masked), 2), dtype=np.float32)
            pvec = np.arange(P, dtype=np.float32)
            for mi, (pr, jt) in enumerate(masked):
                for half in range(2):
                    g = chunks[2 * pr + half]
                    thr[:, mi, half] = jt * P + pvec - g * CHUNK
            im["thr"] = np.ascontiguousarray(thr)
        else:
            # predicate entry (pair, jt): [j=128, i=512] over pair's rows
            pred = np.zeros((len(masked), P, 2 * CHUNK), dtype=BF_NP)
            for i, (pr, jt) in enumerate(masked):
                gA, gB = chunks[2 * pr], chunks[2 * pr + 1]
                rows = np.r_[gA * CHUNK:(gA + 1) * CHUNK,
                             gB * CHUNK:(gB + 1) * CHUNK]
                blk = mask[b, rows, jt * P:(jt + 1) * P]  # [i=512, j=128]
                pred[i] = blk.T.astype(BF_NP)
            im["pred"] = np.ascontiguousarray(pred.transpose(1, 0, 2))
        in_maps.append(im)

    results = run_bass_kernel_spmd(
        nc, in_maps, core_ids=list(range(NCORES)),
        trace=bool(os.environ.get("BASS_TRACE")),
    )
    LAST_RESULTS = results

    out = np.empty((B, S, H), dtype=np.float32)
    for c in range(NCORES):
        b, h = divmod(c, 2)
        chunks = chunks_of[h]
        o = results.results[c]["out"]
        for s, g in enumerate(chunks):
            out[b, g * CHUNK:(g + 1) * CHUNK, :] = \
                o[s * CHUNK:(s + 1) * CHUNK]
    return out


# revision 43
# speedup vs baseline: 2.7364x; 2.7364x over previous
"""Distributed Trainium2 kernel for a single causal attention head.

Problem (hardcoded): B=4, S=2048, D_MODEL=1024, HEAD_DIM=64, fp32 inputs.
    q = query @ Wq + bq ; k = key @ Wk + bk ; v = value @ Wv + bv
    scores = q k^T / sqrt(H) ; masked softmax ; out = att @ v

Sharding (8 NeuronCores): core c = (b, h) with b = c//2, h = c%2.
Each core owns 4 query chunks of 256 rows of batch b and projects the
full k/v of its batch locally.  To balance causal work with one SPMD
program, chunks are assigned h=0 -> {0,3,4,7}, h=1 -> {1,2,5,6}.  The
program works in two slot-PAIRS of 512 query rows; pair p computes
j-tiles [0, SHARED_p) at full 512 width and j-tiles [SHARED_p, SOLO_p)
at 256 width (second chunk only).  Causal: SHARED=(4,12), SOLO=(8,16).
Per-core differences are pure data: gathered query rows and host-built
predicate masks (from the real `mask` input) that zero attention
weights after exp; predicates exist for j-tiles [8p, 8p+8) of pair p.

Device layout trick: query/key/value shards are passed TRANSPOSED and
pre-packed ([128, D/128, S] bf16, one contiguous DMA line per
partition) so every matmul contracts over the partition dim with
operands in natural layout (no on-device input transposes):
  qT[h,i]    = Wq^T Xq^T     (lhsT=Wq chunk, rhs=XqT chunk)
  kT/vT[h,j] = W^T X^T
  v[j,h]     = vT via DRAM bounce + DMA-transpose (2-byte dtype)
  sT[j,i]    = kT-tile as lhsT, rhs=qT         (scores transposed)
  att        = exp(sT * 0.125)   (ScalarE, PSUM->SBUF, bf16)
  oT[65,i]  += v_aug-tile as lhsT, rhs=att     (v_aug ones column ->
                                                row 64 = softmax denom)
Final: PE-transpose [65,128] blocks, scale by reciprocal of column 64,
DMA out as [i, 64] fp32.

DMA discipline (the earlier versions' bottleneck): big wait-free input
DMAs on the sync HWDGE ring in dependency order; the v bounce +
transposes ride the scalar HWDGE ring; output leaves via gpsimd SWDGE.
"""

import os

import numpy as np
import ml_dtypes

import concourse.bass as bass
import concourse.tile as tile
from concourse import bacc, mybir
from concourse.bass import ds
from concourse.bass_utils import run_bass_kernel_spmd
from concourse.masks import make_identity

B, S, D, H = 4, 2048, 1024, 64
P = 128
NCORES = 8
CHUNK = 256               # query rows per chunk/slot
NSLOTS = 4
NPAIRS = 2                # slot pairs; 512 q rows each
NQ = NSLOTS * CHUNK       # 1024
JT = S // P               # 16 j-tiles of 128 keys
DCH = D // P              # 8 contraction chunks
FP = mybir.dt.float32
BF = mybir.dt.bfloat16
U8 = mybir.dt.uint8
BF_NP = ml_dtypes.bfloat16

# (shared extent, solo extent) per pair, in j-tiles; slot extents derive
CAUSAL_PAIRS = ((4, 8), (12, 16))
CAUSAL_CHUNKS = {0: (0, 3, 4, 7), 1: (1, 2, 5, 6)}
# predicate entries: (pair, jt) for jt in [8p, 8p+8)
CAUSAL_MASKED = [(p, jt) for p in range(2) for jt in range(8 * p, 8 * p + 8)]

FULL_PAIRS = ((16, 16), (16, 16))
FULL_CHUNKS = {0: (0, 1, 2, 3), 1: (4, 5, 6, 7)}
FULL_MASKED = [(p, jt) for p in range(2) for jt in range(16)]

LAST_RESULTS = None
_PROGRAM_CACHE = {}


def _build_program(pairs, masked_slots, devpred):
    """Build the SPMD Bass program (identical on all 8 cores).

    devpred=True generates the causal predicate masks on-device from a
    tiny per-core threshold table instead of a 1 MB host predicate DMA.
    """
    nc = bacc.Bacc("TRN2", target_bir_lowering=False, debug=False,
                   num_devices=NCORES)

    qT_d = nc.dram_tensor("qT", [P, DCH, NQ], BF, kind="ExternalInput").ap()
    kT_d = nc.dram_tensor("kT", [P, DCH, S], BF, kind="ExternalInput").ap()
    vT_d = nc.dram_tensor("vT", [P, DCH, S], BF, kind="ExternalInput").ap()
    wall_d = nc.dram_tensor("wall", [P, DCH, 3 * H], BF,
                            kind="ExternalInput").ap()
    ball_d = nc.dram_tensor("ball", [H, 3], FP, kind="ExternalInput").ap()
    nmask = len(masked_slots)
    if devpred:
        thr_d = nc.dram_tensor("thr", [P, nmask, 2], FP,
                               kind="ExternalInput").ap()
    else:
        pred_d = nc.dram_tensor("pred", [P, nmask, 2 * CHUNK], BF,
                                kind="ExternalInput").ap()
    out_d = nc.dram_tensor("out", [NQ, H], FP, kind="ExternalOutput").ap()
    debug = bool(os.environ.get("BASS_DEBUG_DUMP"))
    if debug:
        dbg_k = nc.dram_tensor("dbg_k", [P, S], BF, kind="ExternalOutput").ap()
        dbg_v = nc.dram_tensor("dbg_v", [P, JT, H + 1], BF,
                               kind="ExternalOutput").ap()
        dbg_q = nc.dram_tensor("dbg_q", [P, NQ], BF,
                               kind="ExternalOutput").ap()

    with tile.TileContext(nc) as tc:
        with (
            tc.tile_pool(name="const", bufs=1) as const,
            tc.tile_pool(name="resident", bufs=1) as res,
            tc.tile_pool(name="attp", bufs=26) as attp,
            tc.tile_pool(name="outp", bufs=2) as outp,
            tc.tile_pool(name="dram", bufs=1, space="DRAM") as dram,
            tc.tile_pool(name="pp", bufs=2, space="PSUM") as pp,
            tc.tile_pool(name="psc", bufs=2, space="PSUM") as psc,
            tc.tile_pool(name="pout", bufs=2, space="PSUM") as pout,
            tc.tile_pool(name="ptr", bufs=2, space="PSUM") as ptr,
        ):
            # ---- constants: 2 DMAs on the scalar ring ----
            wall_sb = const.tile([P, DCH, 3 * H], BF, tag="wall")
            nc.scalar.dma_start(wall_sb, wall_d)
            ball_sb = const.tile([H, 3], FP, tag="ball")
            nc.scalar.dma_start(ball_sb, ball_d)
            wk_sb = wall_sb[:, :, 0:H]
            wv_sb = wall_sb[:, :, H:2 * H]
            wq_sb = wall_sb[:, :, 2 * H:3 * H]
            bk_sb = ball_sb[:, 0:1]
            bv_sb = ball_sb[:, 1:2]
            bq_sb = ball_sb[:, 2:3]
            zeros_sb = const.tile([P, 2 * CHUNK], BF, tag="zeros")
            nc.vector.memset(zeros_sb, 0.0)
            ident = const.tile([P, P], FP, tag="ident")
            make_identity(nc, ident)
            identb = const.tile([P, P], BF, tag="identb")
            make_identity(nc, identb)

            # ---- input DMAs on the sync ring, dependency order:
            # k cols 0:1024, q pair0, pred, v cols 0:1024,
            # k cols 1024:2048, v cols 1024:2048, q pair1
            xk_sb = res.tile([P, DCH, S], BF, tag="xk")
            xv_sb = res.tile([P, DCH, S], BF, tag="xv")
            xq_sb = res.tile([P, DCH, NQ], BF, tag="xq")
            pred_sb = res.tile([P, nmask, 2 * CHUNK], BF, tag="pred")

            def dma_cols(eng, dst, src, c0, c1):
                eng.dma_start(dst[:, :, ds(c0, c1 - c0)],
                              src[:, :, ds(c0, c1 - c0)])

            # ONE ring (sync), strict dependency order: transfers drain
            # serially at full HBM rate, so the critical k/q path lands
            # first instead of round-robin-sharing with v.
            dma_cols(nc.sync, xk_sb, kT_d, 0, S // 2)
            dma_cols(nc.sync, xq_sb, qT_d, 0, NQ // 2)
            dma_cols(nc.sync, xq_sb, qT_d, NQ // 2, NQ)
            dma_cols(nc.sync, xv_sb, vT_d, 0, S // 2)
            dma_cols(nc.sync, xk_sb, kT_d, S // 2, S)
            dma_cols(nc.sync, xv_sb, vT_d, S // 2, S)

            if devpred:
                # generate predicates on-device: pred[p, mi, h*256+f] =
                # (f < thr[p, mi, h])
                thr_sb = const.tile([P, nmask, 2], FP, tag="thr")
                nc.scalar.dma_start(thr_sb, thr_d)
                iota_sb = const.tile([P, CHUNK], FP, tag="iota")
                nc.gpsimd.iota(iota_sb, pattern=[[1, CHUNK]], base=0,
                               channel_multiplier=0,
                               allow_small_or_imprecise_dtypes=True)
                for mi in range(nmask):
                    for half in range(2):
                        nc.vector.tensor_scalar(
                            pred_sb[:, mi, ds(half * CHUNK, CHUNK)],
                            iota_sb, thr_sb[:, mi, ds(half, 1)], None,
                            mybir.AluOpType.is_ge)
            else:
                nc.sync.dma_start(pred_sb, pred_d)

            # ---- PE warm-up: keep HAM at full clock until k data lands
            WARM_MMS = 16
            pwarm = pp.tile([P, 512], FP, tag="pp", name="pwarm")
            for _ in range(WARM_MMS):
                nc.tensor.matmul(pwarm, lhsT=identb,
                                 rhs=zeros_sb, start=True, stop=True)

            # col-tiled projection pair: two M=64 matmuls run concurrently
            # in PE column groups 0 / 1, contracting the same weight over
            # two different 512-wide input chunks.
            def proj_pair(w_sb, x_sb, c0, bias, out_fn, name):
                pj = pp.tile([P, 512], FP, tag="pp", name=name)
                for d in range(DCH):
                    nc.tensor.matmul(pj[0:H, :], lhsT=w_sb[:, d, :],
                                     rhs=x_sb[:, d, ds(c0, 512)],
                                     start=(d == 0), stop=(d == DCH - 1),
                                     skip_group_check=True)
                    nc.tensor.matmul(pj[H:2 * H, :], lhsT=w_sb[:, d, :],
                                     rhs=x_sb[:, d, ds(c0 + 512, 512)],
                                     start=(d == 0), stop=(d == DCH - 1),
                                     tile_position=(0, H),
                                     skip_group_check=True)
                out_fn(pj[0:H, :], c0)
                out_fn(pj[H:2 * H, :], c0 + 512)

            k_sb = res.tile([P, S], BF, tag="k")
            nc.vector.memset(k_sb[H:, :], 0.0)
            vT_sb = res.tile([P, S], BF, tag="vT")
            v_sb = res.tile([P, JT, H + 1], BF, tag="v")
            q_sb = res.tile([P, NQ], BF, tag="q")
            nc.vector.memset(q_sb[H:, :], 0.0)

            def k_out(pj, c0):
                nc.scalar.activation(k_sb[:H, ds(c0, 512)], pj,
                                     mybir.ActivationFunctionType.Identity,
                                     bias=bk_sb)

            def v_out(pj, c0):
                nc.scalar.activation(vT_sb[:H, ds(c0, 512)], pj,
                                     mybir.ActivationFunctionType.Identity,
                                     bias=bv_sb)
                for jt in range(c0 // P, c0 // P + 4):
                    pvt = ptr.tile([P, P], BF, tag="tr", name="pvt")
                    nc.tensor.transpose(pvt, vT_sb[:, ds(jt * P, P)], identb)
                    nc.vector.tensor_copy(v_sb[:, jt, 0:H], pvt[:, :H])
                    nc.vector.memset(v_sb[:, jt, H:], 1.0)

            def q_out(pj, c0):
                nc.scalar.activation(q_sb[:H, ds(c0, 512)], pj,
                                     mybir.ActivationFunctionType.Identity,
                                     bias=bq_sb)

            mask_idx = {sj: i for i, sj in enumerate(masked_slots)}
            W = 2 * CHUNK  # 512
            po_tiles = {}
            att_tiles = {}

            def emit_score(pr, jt):
                shared, solo = pairs[pr]
                wide = jt < shared
                c0 = pr * W if wide else pr * W + CHUNK
                n = W if wide else CHUNK
                ps = psc.tile([P, n], FP, tag="sc", name="ps")
                nc.tensor.matmul(ps, lhsT=k_sb[:, ds(jt * P, P)],
                                 rhs=q_sb[:, ds(c0, n)],
                                 start=True, stop=True)
                att = attp.tile([P, n], BF, tag="att", name="att")
                nc.scalar.activation(att, ps,
                                     mybir.ActivationFunctionType.Exp,
                                     scale=0.125)
                mi = mask_idx.get((pr, jt))
                if mi is not None:
                    off = 0 if wide else CHUNK
                    nc.vector.tensor_mul(
                        att, att, pred_sb[:, mi, ds(off, n)])
                att_tiles[(pr, jt)] = (att, c0, n)

            def emit_av(pr, jt):
                solo = pairs[pr][1]
                if pr not in po_tiles:
                    po_tiles[pr] = pout.tile([H + 1, W], FP, tag="po",
                                             name=f"po{pr}")
                att, c0, n = att_tiles.pop((pr, jt))
                nc.tensor.matmul(po_tiles[pr][:, ds(c0 - pr * W, n)],
                                 lhsT=v_sb[:, jt, :], rhs=att,
                                 start=(jt == 0), stop=(jt == solo - 1),
                                 skip_group_check=True)

            out_stage = res.tile([P, NQ // P, H], FP, tag="ostage")

            def epilogue(pr):
                po = po_tiles[pr]
                # transpose + normalize into the staging tile
                oT_sb = outp.tile([P, W], FP, tag="oT")
                nc.vector.tensor_copy(oT_sb[:H + 1, :], po)
                for t in range(W // P):
                    pt = ptr.tile([P, P], FP, tag="tr")
                    nc.tensor.transpose(pt, oT_sb[:, ds(t * P, P)], ident)
                    recip = outp.tile([P, 1], FP, tag="recip")
                    nc.vector.reciprocal(recip, pt[:, H:H + 1])
                    nc.vector.tensor_scalar_mul(
                        out_stage[:, pr * (W // P) + t, :], pt[:, :H], recip)

            # Decoupled emission schedule (per-engine FIFOs are in-order;
            # insert each group where its data will have arrived):
            # fill order: k01@~14, q@~20, k23@~26, v01@~32, v23@~38
            def q_solo(pr):
                pq = pp.tile([H, 512], FP, tag="pp", name="pq")
                for d in range(DCH):
                    nc.tensor.matmul(pq, lhsT=wq_sb[:, d, :],
                                     rhs=xq_sb[:, d, ds(pr * 512, 512)],
                                     start=(d == 0), stop=(d == DCH - 1))
                q_out(pq, pr * 512)

            proj_pair(wk_sb, xk_sb, 0, bk_sb, k_out, "pk01")
            for _ in range(7):    # bridge PE idle until q arrives
                nc.tensor.matmul(pwarm, lhsT=identb, rhs=zeros_sb,
                                 start=True, stop=True)
            q_solo(0)
            q_solo(1)
            # scores for pair0 + pair1's low half, ACT-paced; second-half
            # k projection slots in once k23 has landed
            # fully decoupled: scores+exp groups placed at data-arrival
            # FIFO positions; avs (pure PE, gated on v_sb) fill the gaps
            solo0, solo1 = pairs[0][1], pairs[1][1]
            for jt in range(solo0):
                emit_score(0, jt)
            proj_pair(wv_sb, xv_sb, 0, bv_sb, v_out, "pv01")
            for jt in range(8):
                emit_score(1, jt)
            for jt in range(solo0):
                emit_av(0, jt)
            proj_pair(wk_sb, xk_sb, S // 2, bk_sb, k_out, "pk23")
            for jt in range(8, solo1):
                emit_score(1, jt)
            epilogue(0)
            for jt in range(8):
                emit_av(1, jt)
            proj_pair(wv_sb, xv_sb, S // 2, bv_sb, v_out, "pv23")
            for jt in range(8, solo1):
                emit_av(1, jt)
            epilogue(1)
            nc.gpsimd.dma_start(
                out_d.rearrange("(t p) h -> p t h", p=P), out_stage)

            if debug:
                nc.gpsimd.dma_start(dbg_k, k_sb)
                nc.gpsimd.dma_start(dbg_v, v_sb)
                nc.gpsimd.dma_start(dbg_q, q_sb)

    nc.compile()
    return nc


def _slot_extents(pairs):
    return (pairs[0][0], pairs[0][1], pairs[1][0], pairs[1][1])


def _mask_fits_causal_variant(mask):
    """Causal variant is valid iff, for every chunk, nothing is allowed
    beyond its computed bound and everything below its predicate region
    is allowed."""
    extents = _slot_extents(CAUSAL_PAIRS)
    for h, chunks in CAUSAL_CHUNKS.items():
        for s, g in enumerate(chunks):
            rows = slice(g * CHUNK, (g + 1) * CHUNK)
            bound = extents[s] * P
            lo = (8 * (s // 2)) * P  # predicates cover [8p, 8p+8)
            if bound < S and mask[:, rows, bound:].any():
                return False
            if lo > 0 and not mask[:, rows, :lo].all():
                return False
    return True


def _pack(xT):
    """[D, S] -> [128, D/128, S]: one contiguous DMA line/partition."""
    d, s = xT.shape
    return np.ascontiguousarray(
        xT.reshape(DCH, P, s).transpose(1, 0, 2)).astype(BF_NP)


def _np_reference(query, key, value, mask, Wq, bq, Wk, bk, Wv, bv):
    q = query @ Wq + bq
    k = key @ Wk + bk
    v = value @ Wv + bv
    scores = np.einsum("bqh,bkh->bqk", q, k) / np.sqrt(np.float32(H))
    scores = np.where(mask, scores, np.float32(-1e9))
    scores -= scores.max(axis=-1, keepdims=True)
    e = np.exp(scores)
    att = e / e.sum(axis=-1, keepdims=True)
    return np.einsum("bqk,bkh->bqh", att, v).astype(np.float32)


def kernel(query, key, value, mask, Wq, bq, Wk, bk, Wv, bv):
    global LAST_RESULTS
    query = np.asarray(query, dtype=np.float32)
    key = np.asarray(key, dtype=np.float32)
    value = np.asarray(value, dtype=np.float32)
    mask = np.asarray(mask).astype(bool)
    Wq = np.asarray(Wq, dtype=np.float32)
    Wk = np.asarray(Wk, dtype=np.float32)
    Wv = np.asarray(Wv, dtype=np.float32)
    bq = np.asarray(bq, dtype=np.float32)
    bk = np.asarray(bk, dtype=np.float32)
    bv = np.asarray(bv, dtype=np.float32)

    tril = np.tril(np.ones((S, S), dtype=bool))
    devpred = all(np.array_equal(mask[b], tril) for b in range(B))
    if not devpred:
        # non-causal masks never occur for this problem; fall back to an
        # exact host implementation rather than an untested device path
        return _np_reference(query, key, value, mask, Wq, bq, Wk, bk,
                             Wv, bv)
    pairs, chunks_of, masked = CAUSAL_PAIRS, CAUSAL_CHUNKS, CAUSAL_MASKED
    key_v = ("causal", True)

    if key_v not in _PROGRAM_CACHE:
        _PROGRAM_CACHE[key_v] = _build_program(pairs, masked, True)
    nc = _PROGRAM_CACHE[key_v]

    def packw(w):
        return np.ascontiguousarray(
            w.reshape(DCH, P, H).transpose(1, 0, 2)).astype(BF_NP)

    # weight layout must match the wall_sb slicing: wk | wv | wq
    wall_in = np.concatenate([packw(Wk), packw(Wv), packw(Wq)], axis=2)
    wall_in = np.ascontiguousarray(wall_in)
    ball_in = np.ascontiguousarray(
        np.stack([bk, bv, bq], axis=1).astype(np.float32))

    in_maps = []
    for c in range(NCORES):
        b, h = divmod(c, 2)
        chunks = chunks_of[h]
        q_rows = np.concatenate(
            [query[b, g * CHUNK:(g + 1) * CHUNK, :] for g in chunks], axis=0)
        qT = _pack(q_rows.T)
        kT = _pack(key[b].T)
        vT = _pack(value[b].T)
        im = {"qT": qT, "kT": kT, "vT": vT,
              "wall": wall_in, "ball": ball_in}
        if devpred:
            # threshold table: pred[p, mi, half*256+f] = (f < thr)
            thr = np.zeros((P, len(masked), 2), dtype=np.float32)
            pvec = np.arange(P, dtype=np.float32)
            for mi, (pr, jt) in enumerate(masked):
                for half in range(2):
                    g = chunks[2 * pr + half]
                    thr[:, mi, half] = jt * P + pvec - g * CHUNK
            im["thr"] = np.ascontiguousarray(thr)
        else:
            # predicate entry (pair, jt): [j=128, i=512] over pair's rows
            pred = np.zeros((len(masked), P, 2 * CHUNK), dtype=BF_NP)
            for i, (pr, jt) in enumerate(masked):
                gA, gB = chunks[2 * pr], chunks[2 * pr + 1]
                rows = np.r_[gA * CHUNK:(gA + 1) * CHUNK,
                             gB * CHUNK:(gB + 1) * CHUNK]
                blk = mask[b, rows, jt * P:(jt + 1) * P]  # [i=512, j=128]
                pred[i] = blk.T.astype(BF_NP)
            im["pred"] = np.ascontiguousarray(pred.transpose(1, 0, 2))
        in_maps.append(im)

    results = run_bass_kernel_spmd(
        nc, in_maps, core_ids=list(range(NCORES)),
        trace=bool(os.environ.get("BASS_TRACE")),
    )
    LAST_RESULTS = results

    out = np.empty((B, S, H), dtype=np.float32)
    for c in range(NCORES):
        b, h = divmod(c, 2)
        chunks = chunks_of[h]
        o = results.results[c]["out"]
        for s, g in enumerate(chunks):
            out[b, g * CHUNK:(g + 1) * CHUNK, :] = \
                o[s * CHUNK:(s + 1) * CHUNK]
    return out
